# revision 15
# baseline (speedup 1.0000x reference)
"""Luong seq2seq (2-layer BiGRU encoder + attention GRU decoder + vocab
projection) as an 8-core SPMD Bass/Tile kernel for Trainium2.

v2: everything runs in feature-major ("transposed") layout — hidden/gate
features live in the partition dim, (time x batch) lives in the free dim.
This removes all per-step PE transposes, makes the elementwise GRU chain
cheap (free dim = 16/8 instead of 512), and turns every weight apply into
128x128 bf16 stationary matmuls (fast weight load) with small moving
operands.  The decoder uses tanh-based sigmoids so the whole decode phase
stays on the exp/tanh activation-table set (no per-step table reloads).

Sharding: data-parallel over batch (64 examples -> 8 per core); each core
projects onto the full 32000-word vocab; host concatenates.

Self-contained: hardcodes all shapes; takes full unsharded inputs and
returns the full (48, 64, 32000) f32 logits.
"""

import os
import sys
import types

for _p in ("/opt/trn_rl_repo", "/opt/pypackages", "/root/.axon_site",
           "/root/.axon_site/_ro/trn_rl_repo", "/root/.axon_site/_ro/pypackages"):
    if os.path.isdir(_p) and _p not in sys.path:
        sys.path.append(_p)

import numpy as np

from concourse import bass, mybir, tile, bacc
from concourse import bass_utils
from concourse.bass_utils import run_bass_kernel_spmd
from concourse.masks import make_identity

# ---------------------------------------------------------------- constants
V, H, T, B, NCORES = 32000, 512, 48, 64, 8
Bc = B // NCORES            # 8 examples per core
H2, H3 = 2 * H, 3 * H
NSEQ = T * Bc               # 384 (r = t*8 + b)
NSCAN = T * 2 * Bc          # 768 (r = s*16 + dir*8 + b)
P = 128
NEG = -1.0e9

f32 = mybir.dt.float32
f32r = mybir.dt.float32r
bf16 = mybir.dt.bfloat16
AF = mybir.ActivationFunctionType
OP = mybir.AluOpType

VCHUNKS = [(i * 1024, 1024) for i in range(31)] + [(31744, 256)]  # 32000


def _install_profile_hook():
    """Make trace=True work: the image's antenv lacks axon_hooks."""
    if "antenv.axon_hooks" in sys.modules:
        return
    try:
        import trn_agent_boot.trn_boot as tb
        hook = tb._ntff_profile_via_ctypes("/opt/axon/libaxon_pjrt.so")
        m = types.ModuleType("antenv.axon_hooks")
        m.get_axon_ntff_profile_hook = lambda: hook
        m.set_axon_ntff_profile_hook = lambda h: None
        sys.modules["antenv.axon_hooks"] = m
        import antenv
        antenv.axon_hooks = m
        bass_utils.upload_artifacts = lambda d: d
    except Exception:
        pass


# ---------------------------------------------------------------- program
def build_program(dbg=False):
    nc = bacc.Bacc("TRN2", target_bir_lowering=False, debug=False,
                   num_devices=NCORES)

    def din(name, shape, dt=f32r):
        return nc.dram_tensor(name, list(shape), dt, kind="ExternalInput").ap()

    io = {}
    io["xeT_in"] = din("xeT_in", (H, NSCAN), bf16)
    io["xdT_in"] = din("xdT_in", (H, NSEQ), bf16)
    io["amaskT"] = din("amaskT", (NSEQ, Bc), f32)
    for name, shape in [
        ("w0", (H, H3)), ("u0", (H, H3)),
        ("w1", (H2, H3)), ("u1", (H, H3)),
        ("wxd", (H, H3)), ("whd", (H, H3)), ("ud", (H, H3)),
        ("wa", (H2, H)), ("wcc", (H2, H)), ("wch", (H, H)),
        ("fct", (H2, H)),
        ("bn0", (1, H)), ("bn1", (1, H)), ("bnd", (1, H)),
    ]:
        io[name] = din(name, shape, bf16)
    io["b0"] = din("b0", (P, 12), f32)
    io["b1"] = din("b1", (P, 12), f32)
    io["bd"] = din("bd", (P, 12), f32)
    io["fcb"] = din("fcb", (P, 4), f32)
    io["owt"] = din("owt", (H, V), bf16)
    io["out"] = nc.dram_tensor("out", [NSEQ, V], bf16,
                               kind="ExternalOutput").ap()
    io["dbg"] = dbg
    if dbg:
        io["dbg_l0T"] = nc.dram_tensor("dbg_l0T", [P, 8, NSCAN], bf16,
                                       kind="ExternalOutput").ap()
        io["dbg_hencT"] = nc.dram_tensor("dbg_hencT", [P, 8, NSEQ], bf16,
                                         kind="ExternalOutput").ap()
        io["dbg_h0T"] = nc.dram_tensor("dbg_h0T", [P, 4, Bc], bf16,
                                       kind="ExternalOutput").ap()
        io["dbg_htT"] = nc.dram_tensor("dbg_htT", [P, 4, NSEQ], bf16,
                                       kind="ExternalOutput").ap()
        io["dbg_hall"] = nc.dram_tensor("dbg_hall", [P, 4, NSEQ], bf16,
                                        kind="ExternalOutput").ap()
        io["dbg_sc"] = nc.dram_tensor("dbg_sc", [P, 3, NSEQ], f32,
                                      kind="ExternalOutput").ap()

    with tile.TileContext(nc) as tc:
        _emit(nc, tc, io)
    nc.compile()
    return nc


def _emit(nc, tc, io):
    # ---------------- pools
    cpool_cm = tc.tile_pool(name="const", bufs=1)
    spool_cm = tc.tile_pool(name="state", bufs=2)
    gpool_cm = tc.tile_pool(name="gch", bufs=3)      # chain temporaries
    big_cm = tc.tile_pool(name="big", bufs=1)        # big sequence tiles
    wseq_cm = tc.tile_pool(name="wseq", bufs=1)      # phase-scoped weights
    prp_cm = tc.tile_pool(name="proj", bufs=4)
    pbank_cm = tc.tile_pool(name="pbank", bufs=1, space="PSUM")
    ppj_cm = tc.tile_pool(name="ppj", bufs=2, space="PSUM")
    cpool = cpool_cm.__enter__()
    spool = spool_cm.__enter__()
    gpool = gpool_cm.__enter__()
    big = big_cm.__enter__()
    wseq = wseq_cm.__enter__()
    prp = prp_cm.__enter__()
    pbank = pbank_cm.__enter__()
    ppj = ppj_cm.__enter__()

    # ---------------- constants
    identf = cpool.tile([P, P], f32)
    make_identity(nc, identf[:])
    identb = cpool.tile([P, P], bf16)
    nc.vector.tensor_copy(identb[:], identf[:])
    ones_c = cpool.tile([1, P], bf16)       # lhsT for bcast (K=1, M=128)
    nc.vector.memset(ones_c[:], 1.0)
    ones_r = cpool.tile([P, 1], bf16)       # lhsT for col-sum (K=128, M=1)
    nc.vector.memset(ones_r[:], 1.0)
    ones16 = cpool.tile([1, 16], bf16)      # rhs for bias injects
    nc.vector.memset(ones16[:], 1.0)

    def load(pool, name, shape, dt, tag, rearr=None):
        t = pool.tile(list(shape), dt, tag=tag, name=name + "_sb")
        src = io[name]
        if rearr is not None:
            src = src.rearrange(rearr, p=P)
            nc.sync.dma_start(t[:, :, :shape[2]] if len(shape) == 3 else t[:],
                              src)
        else:
            nc.sync.dma_start(t[:], src[:])
        return t

    bn0_sb = load(cpool, "bn0", (1, H), bf16, "bn0")
    bn1_sb = load(cpool, "bn1", (1, H), bf16, "bn1")
    bnd_sb = load(cpool, "bnd", (1, H), bf16, "bnd")
    b0_sb = load(cpool, "b0", (P, 12), f32, "b0")
    b1_sb = load(cpool, "b1", (P, 12), f32, "b1")
    bd_sb = load(cpool, "bd", (P, 12), f32, "bd")
    fcb_sb = load(cpool, "fcb", (P, 4), f32, "fcb")
    amask_sb = load(cpool, "amaskT", (P, 3, Bc), f32, "amaskT",
                    "(c p) b -> p c b")

    # PSUM banks (each padded to a full 2KB bank)
    ps_r = pbank.tile([P, 4, 16], f32, padded_shape=[P, 4, P], tag="bk_r")
    ps_z = pbank.tile([P, 4, 16], f32, padded_shape=[P, 4, P], tag="bk_z")
    ps_n = pbank.tile([P, 4, 32], f32, padded_shape=[P, 4, P], tag="bk_n")
    ps_sc = pbank.tile([P, 4, P], f32, tag="bk_sc")
    ps_ht = pbank.tile([P, 4, 16], f32, padded_shape=[P, 4, P], tag="bk_ht")
    ps_s = pbank.tile([P, 32], f32, padded_shape=[P, 512], tag="bk_s")

    def bank_mms(ps_g, nw, groups):
        """Emit per-region accumulation groups into one PSUM bank.

        groups: list of regions; each region is a list of (m, off, lhsT,
        rhs) mms accumulated into ps_g[:, m, off:off+nw].  Each region is a
        contiguous accumulation group (start on its first mm, stop on its
        last), the only pattern valid under both per-element (HW) and
        whole-bank (CoreSim) has_written-clear semantics."""
        for grp in groups:
            last = len(grp) - 1
            for i, (m, off, lh, rh) in enumerate(grp):
                nc.tensor.matmul(ps_g[:, m, off:off + nw], lh, rh,
                                 start=(i == 0), stop=(i == last),
                                 skip_group_check=True)

    def warm_mm(rhs):
        nc.tensor.matmul(ps_sc[:, 3, 0:P], identb[:], rhs,
                         start=True, stop=True, skip_group_check=True)

    # ---------------- pre-projection GEMM -> bf16 xp tile (+ bias)
    def pre_gemm(xp_t, w_sb, ko, rhs, ncols, bias_sb):
        segs = [(s, min(384, ncols - s)) for s in range(0, ncols, 384)]
        for c in range(12):
            for s0, sw in segs:
                pp = ppj.tile([P, 512], f32, tag="gemm", name="pp")
                for k in range(ko):
                    nc.tensor.matmul(pp[:, :sw],
                                     w_sb[:, k, c * P:(c + 1) * P],
                                     rhs[:, k, s0:s0 + sw],
                                     start=(k == 0), stop=(k == ko - 1))
                nc.scalar.activation(xp_t[:, c, s0:s0 + sw], pp[:, :sw],
                                     AF.Identity, bias=bias_sb[:, c:c + 1])

    # ================================================= input staging
    xeT = load(big, "xeT_in", (P, 4, NSCAN), bf16, "xeT", "(k p) n -> p k n")
    xdT = load(big, "xdT_in", (P, 4, NSEQ), bf16, "xdT", "(k p) n -> p k n")

    w0_sb = load(wseq, "w0", (P, 4, H3), bf16, "wqA", "(k p) n -> p k n")
    wxd_sb = load(wseq, "wxd", (P, 4, H3), bf16, "wqB", "(k p) n -> p k n")

    xp0 = big.tile([P, 12, NSCAN], bf16, tag="xp0")
    xpx = big.tile([P, 12, NSEQ], bf16, tag="xpx")
    pre_gemm(xp0, w0_sb, 4, xeT, NSCAN, b0_sb)
    pre_gemm(xpx, wxd_sb, 4, xdT, NSEQ, bd_sb)

    l0T = big.tile([P, 8, NSCAN], bf16, tag="l0T")
    hencT = big.tile([P, 8, NSEQ], bf16, tag="hencT")

    # ================================================= GRU scan (encoder)
    def enc_scan(u_sb, bn_sb, xp_t, emit):
        h = None
        for s in range(T):
            c0 = s * 16
            for g, ps_g in ((0, ps_r), (2, ps_n), (1, ps_z)):
                groups = []
                for m in range(4):
                    c = g * 4 + m
                    grp = []
                    if g == 2:
                        grp.append((m, 0, bn_sb[:, m * P:(m + 1) * P],
                                    ones16[:, :]))
                    else:
                        grp.append((m, 0, identb[:], xp_t[:, c, c0:c0 + 16]))
                    if h is not None:
                        for k in range(4):
                            grp.append((m, 0, u_sb[:, k, c * P:(c + 1) * P],
                                        h[:, k, :]))
                    groups.append(grp)
                bank_mms(ps_g, 16, groups)
            warm_mm(xp_t[:, 0, 0:P])
            # ---- chain
            r = gpool.tile([P, 4, 16], f32, tag="gr", name="r")
            nc.scalar.activation(r[:], ps_r[:, :, 0:16], AF.Sigmoid)
            u_t = gpool.tile([P, 4, 16], f32, tag="gu", name="u_t")
            nc.vector.tensor_mul(u_t[:], r[:], ps_n[:, :, 0:16])
            v_t = gpool.tile([P, 4, 16], f32, tag="gv", name="v_t")
            nc.vector.tensor_add(v_t[:], u_t[:], xp_t[:, 8:12, c0:c0 + 16])
            n_t = gpool.tile([P, 4, 16], f32, tag="gn", name="n_t")
            nc.scalar.activation(n_t[:], v_t[:], AF.Tanh)
            z_t = gpool.tile([P, 4, 16], f32, tag="gz", name="z_t")
            nc.scalar.activation(z_t[:], ps_z[:, :, 0:16], AF.Sigmoid)
            h_new = spool.tile([P, 4, 16], bf16, tag="h", name="h_new")
            d_t = gpool.tile([P, 4, 16], f32, tag="gd", name="d_t")
            if h is not None:
                nc.vector.tensor_sub(d_t[:], h[:], n_t[:])
                e_t = gpool.tile([P, 4, 16], f32, tag="ge", name="e_t")
                nc.vector.tensor_mul(e_t[:], z_t[:], d_t[:])
                nc.vector.tensor_add(h_new[:], n_t[:], e_t[:])
            else:
                nc.vector.tensor_mul(d_t[:], z_t[:], n_t[:])
                nc.vector.tensor_sub(h_new[:], n_t[:], d_t[:])
            emit(s, h_new)
            h = h_new

    def emit_l0(s, h_new):
        sr = T - 1 - s
        nc.scalar.copy(out=l0T[:, 0:4, s * 16:s * 16 + 8],
                       in_=h_new[:, :, 0:8])
        nc.scalar.copy(out=l0T[:, 4:8, s * 16 + 8:s * 16 + 16],
                       in_=h_new[:, :, 8:16])
        nc.vector.tensor_copy(out=l0T[:, 0:4, sr * 16 + 8:sr * 16 + 16],
                              in_=h_new[:, :, 0:8])
        nc.vector.tensor_copy(out=l0T[:, 4:8, sr * 16:sr * 16 + 8],
                              in_=h_new[:, :, 8:16])

    def emit_henc(s, h_new):
        sr = T - 1 - s
        nc.scalar.copy(out=hencT[:, 0:4, s * Bc:(s + 1) * Bc],
                       in_=h_new[:, :, 0:8])
        nc.vector.tensor_copy(out=hencT[:, 4:8, sr * Bc:(sr + 1) * Bc],
                              in_=h_new[:, :, 8:16])

    u0_sb = load(wseq, "u0", (P, 4, H3), bf16, "wqA", "(k p) n -> p k n")
    enc_scan(u0_sb, bn0_sb, xp0, emit_l0)

    w1_sb = load(wseq, "w1", (P, 8, H3), bf16, "wqC", "(k p) n -> p k n")
    xp1 = big.tile([P, 12, NSCAN], bf16, tag="xp0", name="xp1")
    pre_gemm(xp1, w1_sb, 8, l0T, NSCAN, b1_sb)

    u1_sb = load(wseq, "u1", (P, 4, H3), bf16, "wqA", "(k p) n -> p k n")
    enc_scan(u1_sb, bn1_sb, xp1, emit_henc)

    if io["dbg"]:
        nc.sync.dma_start(io["dbg_l0T"][:], l0T[:])
        nc.sync.dma_start(io["dbg_hencT"][:], hencT[:])

    # ================================================= attention precompute
    wa_sb = load(wseq, "wa", (P, 8, H), bf16, "wqD", "(k p) n -> p k n")
    gT = big.tile([P, 4, NSEQ], bf16, tag="gT")
    for m in range(4):
        pp = ppj.tile([P, 512], f32, tag="gemm", name="pp")
        for k in range(8):
            nc.tensor.matmul(pp[:, :NSEQ], wa_sb[:, k, m * P:(m + 1) * P],
                             hencT[:, k, :], start=(k == 0), stop=(k == 7))
        nc.scalar.copy(out=gT[:, m, :], in_=pp[:, :NSEQ])

    wcc_sb = load(wseq, "wcc", (P, 8, H), bf16, "wqD", "(k p) n -> p k n")
    pf = big.tile([P, 3, H], bf16, tag="pf")
    for m in range(3):
        pp = ppj.tile([P, 512], f32, tag="gemm", name="pp")
        for k in range(8):
            nc.tensor.matmul(pp[:, :H], hencT[:, k, m * P:(m + 1) * P],
                             wcc_sb[:, k, :], start=(k == 0), stop=(k == 7))
        nc.vector.tensor_copy(out=pf[:, m, :], in_=pp[:, :H])

    # h0 = tanh(fc_init([hf; hb]) + fcb)
    fct_sb = load(wseq, "fct", (P, 8, H), bf16, "wqD", "(k p) n -> p k n")
    groups = []
    for m in range(4):
        grp = []
        for k in range(8):
            rhs = (hencT[:, k, (T - 1) * Bc:T * Bc] if k < 4
                   else hencT[:, k, 0:Bc])
            grp.append((m, 0, fct_sb[:, k, m * P:(m + 1) * P], rhs))
        groups.append(grp)
    bank_mms(ps_ht, Bc, groups)
    h0T = spool.tile([P, 4, Bc], bf16, tag="hd", name="h0T")
    for m in range(4):
        nc.scalar.activation(h0T[:, m, :], ps_ht[:, m, 0:Bc], AF.Tanh,
                             bias=fcb_sb[:, m:m + 1])

    # ================================================= decoder
    ud_sb = load(wseq, "ud", (P, 4, H3), bf16, "wqA", "(k p) n -> p k n")
    whd_sb = load(wseq, "whd", (P, 4, H3), bf16, "wqB", "(k p) n -> p k n")
    wch_sb = load(wseq, "wch", (P, 4, H), bf16, "wqE", "(k p) n -> p k n")
    htall = big.tile([P, 4, NSEQ], bf16, tag="htall")
    owt_r = io["owt"].rearrange("(k p) v -> p k v", p=P)

    def emit_proj(m, c0, cw):
        ow = prp.tile([P, 4, 1024], bf16, tag="ow", name="ow")
        nc.sync.dma_start(ow[:, :, :cw], owt_r[:, :, c0:c0 + cw])
        ob = prp.tile([P, 1024], bf16, tag="ob", name="ob")
        for h0 in range(0, cw, 512):
            hw = min(512, cw - h0)
            pp = ppj.tile([P, 512], f32, tag="gemm", name="pp")
            for k in range(4):
                nc.tensor.matmul(pp[:, :hw],
                                 htall[:, k, m * P:(m + 1) * P],
                                 ow[:, k, h0:h0 + hw],
                                 start=(k == 0), stop=(k == 3))
            nc.scalar.copy(out=ob[:, h0:h0 + hw], in_=pp[:, :hw])
        nc.sync.dma_start(io["out"][m * P:(m + 1) * P, c0:c0 + cw],
                          ob[:, :cw])

    todo = {m: list(VCHUNKS) for m in range(3)}
    if io["dbg"]:
        hall_d = big.tile([P, 4, NSEQ], bf16, tag="halld")
        scall_d = big.tile([P, 3, NSEQ], f32, tag="scalld")

    hT = h0T
    htT = None
    for t in range(T):
        c0 = t * Bc
        # ---- gates
        for g, ps_g in ((0, ps_r), (2, ps_n), (1, ps_z)):
            groups = []
            if g == 2:
                # nx (outside r*(.)) regions first: x-part inject + ht-part
                for m in range(4):
                    c = g * 4 + m
                    grp = [(m, 16, identb[:], xpx[:, c, c0:c0 + Bc])]
                    if htT is not None:
                        for k in range(4):
                            grp.append((m, 16,
                                        whd_sb[:, k, c * P:(c + 1) * P],
                                        htT[:, k, :]))
                    groups.append(grp)
            for m in range(4):
                c = g * 4 + m
                if g == 2:
                    grp = [(m, 0, bnd_sb[:, m * P:(m + 1) * P],
                            ones16[:, 0:Bc])]
                else:
                    grp = [(m, 0, identb[:], xpx[:, c, c0:c0 + Bc])]
                for k in range(4):
                    grp.append((m, 0, ud_sb[:, k, c * P:(c + 1) * P],
                                hT[:, k, :]))
                if htT is not None and g != 2:
                    for k in range(4):
                        grp.append((m, 0, whd_sb[:, k, c * P:(c + 1) * P],
                                    htT[:, k, :]))
                groups.append(grp)
            bank_mms(ps_g, Bc, groups)
        # ---- GRU chain (tanh-sigmoid keeps decoder on the exp/tanh set)
        thr = gpool.tile([P, 4, Bc], f32, tag="gr", name="thr")
        nc.scalar.activation(thr[:], ps_r[:, :, 0:Bc], AF.Tanh, scale=0.5)
        u_t = gpool.tile([P, 4, Bc], f32, tag="gu", name="u_t")
        nc.vector.scalar_tensor_tensor(u_t[:], thr[:], 1.0,
                                       ps_n[:, :, 0:Bc], OP.add, OP.mult)
        v_t = gpool.tile([P, 4, Bc], f32, tag="gv", name="v_t")
        nc.vector.scalar_tensor_tensor(v_t[:], u_t[:], 0.5,
                                       ps_n[:, :, 16:16 + Bc],
                                       OP.mult, OP.add)
        n_t = gpool.tile([P, 4, Bc], f32, tag="gn", name="n_t")
        nc.scalar.activation(n_t[:], v_t[:], AF.Tanh)
        thz = gpool.tile([P, 4, Bc], f32, tag="gz", name="thz")
        nc.scalar.activation(thz[:], ps_z[:, :, 0:Bc], AF.Tanh, scale=0.5)
        d_t = gpool.tile([P, 4, Bc], f32, tag="gd", name="d_t")
        nc.vector.tensor_sub(d_t[:], hT[:], n_t[:])
        e_t = gpool.tile([P, 4, Bc], f32, tag="ge", name="e_t")
        nc.vector.scalar_tensor_tensor(e_t[:], thz[:], 1.0, d_t[:],
                                       OP.add, OP.mult)
        h_new = spool.tile([P, 4, Bc], bf16, tag="hd", name="h_new")
        nc.vector.scalar_tensor_tensor(h_new[:], e_t[:], 0.5, n_t[:],
                                       OP.mult, OP.add)
        hT = h_new
        if io["dbg"]:
            nc.vector.tensor_copy(hall_d[:, :, c0:c0 + Bc], h_new[:])
        # ---- attention: scores (ps_sc), wch part (ps_ht, early)
        groups = []
        for mt in range(3):
            groups.append([(mt, 0, gT[:, k, mt * P:(mt + 1) * P],
                            hT[:, k, :]) for k in range(4)])
        bank_mms(ps_sc, Bc, groups)
        # ht bank: per-m contiguous groups [wch k0..3, ctx j0..2]; the
        # ctx part is emitted below once alphaT exists
        scm = gpool.tile([P, 3, Bc], f32, tag="gsc", name="scm")
        nc.vector.tensor_add(scm[:], ps_sc[:, 0:3, 0:Bc], amask_sb[:])
        if io["dbg"]:
            nc.vector.tensor_copy(scall_d[:, :, c0:c0 + Bc], scm[:])
        expT = gpool.tile([P, 3, Bc], bf16, tag="gexp", name="expT")
        nc.scalar.activation(expT[:], scm[:], AF.Exp)
        # S = sum_t exp (per example), via ones col-sum matmuls
        for j in range(3):
            nc.tensor.matmul(ps_s[0:1, 0:Bc], ones_r[:],
                             expT[:, j, :], start=(j == 0), stop=(j == 2),
                             skip_group_check=True)
        rS = gpool.tile([1, Bc], bf16, tag="grs", name="rS")
        with nc.allow_low_precision(reason="bf16 1/S is plenty for softmax"):
            nc.vector.reciprocal(rS[:], ps_s[0:1, 0:Bc])
        # broadcast 1/S across partitions via PE
        nc.tensor.matmul(ps_s[:, 16:16 + Bc], ones_c[:], rS[:],
                         start=True, stop=True, skip_group_check=True)
        alphaT = gpool.tile([P, 3, Bc], bf16, tag="galp", name="alphaT")
        for j in range(3):
            nc.vector.tensor_mul(alphaT[:, j, :], expT[:, j, :],
                                 ps_s[:, 16:16 + Bc])
        groups = []
        for m in range(4):
            grp = [(m, 0, wch_sb[:, k, m * P:(m + 1) * P], hT[:, k, :])
                   for k in range(4)]
            grp += [(m, 0, pf[:, j, m * P:(m + 1) * P], alphaT[:, j, :])
                    for j in range(3)]
            groups.append(grp)
        bank_mms(ps_ht, Bc, groups)
        ht_new = spool.tile([P, 4, Bc], bf16, tag="htd", name="ht_new")
        nc.scalar.activation(ht_new[:], ps_ht[:, :, 0:Bc], AF.Tanh)
        nc.vector.tensor_copy(out=htall[:, :, c0:c0 + Bc], in_=ht_new[:])
        htT = ht_new
        # ---- interleaved vocab projection
        if t >= 17:
            m = min(2, (t - 17) // 16)
            for _ in range(3):
                if todo[m]:
                    cc0, cw = todo[m].pop(0)
                    emit_proj(m, cc0, cw)

    for m in range(3):
        while todo[m]:
            cc0, cw = todo[m].pop(0)
            emit_proj(m, cc0, cw)

    if io["dbg"]:
        nc.sync.dma_start(io["dbg_h0T"][:], h0T[:])
        nc.sync.dma_start(io["dbg_htT"][:], htall[:])
        nc.sync.dma_start(io["dbg_hall"][:], hall_d[:])
        nc.sync.dma_start(io["dbg_sc"][:], scall_d[:])

    for cm in (ppj_cm, pbank_cm, prp_cm, wseq_cm, big_cm, gpool_cm,
               spool_cm, cpool_cm):
        cm.__exit__(None, None, None)


# ---------------------------------------------------------------- host side
_PROGRAM = None


def _get_program():
    global _PROGRAM
    if _PROGRAM is None:
        _install_profile_hook()
        _PROGRAM = build_program()
    return _PROGRAM


def _prep_shared(inputs):
    import ml_dtypes
    f = np.float32
    bf = ml_dtypes.bfloat16

    def cat_u(pre):
        return np.concatenate(
            [np.asarray(inputs[f"{pre}_Ur"], f).T,
             np.asarray(inputs[f"{pre}_Uz"], f).T,
             np.asarray(inputs[f"{pre}_Un"], f).T], axis=1)

    g = {}
    g["w0"] = np.asarray(inputs["enc0_Wih"], f).T.astype(bf)
    g["u0"] = cat_u("enc0").astype(bf)
    g["w1"] = np.asarray(inputs["enc1_Wih"], f).T.astype(bf)
    g["u1"] = cat_u("enc1").astype(bf)
    dwih = np.asarray(inputs["dec_Wih"], f)
    g["wxd"] = dwih[:, :H].T.astype(bf)
    g["whd"] = dwih[:, H:].T.astype(bf)
    g["ud"] = cat_u("dec").astype(bf)
    scale = np.float32(1.0) / np.sqrt(np.float32(H2))
    g["wa"] = (np.asarray(inputs["Wa"], f) * scale).astype(bf)
    acw = np.asarray(inputs["attn_combine_w"], f)
    g["wch"] = acw[:, :H].T.astype(bf)
    g["wcc"] = acw[:, H:].T.astype(bf)
    g["fct"] = np.asarray(inputs["fc_init_w"], f).T.astype(bf)
    g["bn0"] = np.asarray(inputs["enc0_bn"], f)[None, :].astype(bf)
    g["bn1"] = np.asarray(inputs["enc1_bn"], f)[None, :].astype(bf)
    g["bnd"] = np.asarray(inputs["dec_bn"], f)[None, :].astype(bf)
    g["b0"] = np.asarray(inputs["enc0_bih"], f).reshape(12, P).T
    g["b1"] = np.asarray(inputs["enc1_bih"], f).reshape(12, P).T
    g["bd"] = np.asarray(inputs["dec_bih"], f).reshape(12, P).T
    g["fcb"] = np.asarray(inputs["fc_init_b"], f).reshape(4, P).T
    g["owt"] = np.asarray(inputs["out_w"], f).T.astype(bf)
    for k in g:
        g[k] = np.ascontiguousarray(g[k])
    return g


def _prep_core(inputs, c):
    src = np.asarray(inputs["src"])
    tgt = np.asarray(inputs["tgt"])
    emb = np.asarray(inputs["emb"], np.float32)
    import ml_dtypes
    bf = ml_dtypes.bfloat16
    si = src[:, c * Bc:(c + 1) * Bc].astype(np.int64)      # (48, 8)
    ti = tgt[:, c * Bc:(c + 1) * Bc].astype(np.int64)
    idx_enc = np.empty((T, 2, Bc), np.int64)
    idx_enc[:, 0, :] = si
    idx_enc[:, 1, :] = si[::-1]
    xeT_in = np.ascontiguousarray(emb[idx_enc.reshape(NSCAN)].T.astype(bf))
    xdT_in = np.ascontiguousarray(emb[ti.reshape(NSEQ)].T.astype(bf))
    m = np.full((T, Bc, Bc), NEG, np.float32)
    for b in range(Bc):
        m[:, b, b] = np.where(si[:, b] != 0, np.float32(0.0),
                              np.float32(NEG))
    return {"xeT_in": xeT_in,
            "xdT_in": xdT_in,
            "amaskT": np.ascontiguousarray(m.reshape(NSEQ, Bc))}


def kernel(**inputs):
    nc = _get_program()
    shared = _prep_shared(inputs)
    in_maps = []
    for c in range(NCORES):
        im = dict(shared)
        im.update(_prep_core(inputs, c))
        in_maps.append(im)
    res = run_bass_kernel_spmd(nc, in_maps, core_ids=list(range(NCORES)))
    logits = np.empty((T, B, V), np.float32)
    for c in range(NCORES):
        logits[:, c * Bc:(c + 1) * Bc, :] = \
            res.results[c]["out"].astype(np.float32).reshape(T, Bc, V)
    return logits


# revision 17
# speedup vs baseline: 1.0997x; 1.0997x over previous
"""Luong seq2seq (2-layer BiGRU encoder + attention GRU decoder + vocab
projection) as an 8-core SPMD Bass/Tile kernel for Trainium2.

v2: everything runs in feature-major ("transposed") layout — hidden/gate
features live in the partition dim, (time x batch) lives in the free dim.
This removes all per-step PE transposes, makes the elementwise GRU chain
cheap (free dim = 16/8 instead of 512), and turns every weight apply into
128x128 bf16 stationary matmuls (fast weight load) with small moving
operands.  The decoder uses tanh-based sigmoids so the whole decode phase
stays on the exp/tanh activation-table set (no per-step table reloads).

Sharding: data-parallel over batch (64 examples -> 8 per core); each core
projects onto the full 32000-word vocab; host concatenates.

Self-contained: hardcodes all shapes; takes full unsharded inputs and
returns the full (48, 64, 32000) f32 logits.
"""

import os
import sys
import types

for _p in ("/opt/trn_rl_repo", "/opt/pypackages", "/root/.axon_site",
           "/root/.axon_site/_ro/trn_rl_repo", "/root/.axon_site/_ro/pypackages"):
    if os.path.isdir(_p) and _p not in sys.path:
        sys.path.append(_p)

import numpy as np

from concourse import bass, mybir, tile, bacc
from concourse import bass_utils
from concourse.bass_utils import run_bass_kernel_spmd
from concourse.masks import make_identity

# ---------------------------------------------------------------- constants
V, H, T, B, NCORES = 32000, 512, 48, 64, 8
Bc = B // NCORES            # 8 examples per core
H2, H3 = 2 * H, 3 * H
NSEQ = T * Bc               # 384 (r = t*8 + b)
NSCAN = T * 2 * Bc          # 768 (r = s*16 + dir*8 + b)
P = 128
NEG = -1.0e9

f32 = mybir.dt.float32
f32r = mybir.dt.float32r
bf16 = mybir.dt.bfloat16
AF = mybir.ActivationFunctionType
OP = mybir.AluOpType

VCHUNKS = [(i * 512, 512) for i in range(62)] + [(62 * 512, 256)]  # 32000


def _install_profile_hook():
    """Make trace=True work: the image's antenv lacks axon_hooks."""
    if "antenv.axon_hooks" in sys.modules:
        return
    try:
        import trn_agent_boot.trn_boot as tb
        hook = tb._ntff_profile_via_ctypes("/opt/axon/libaxon_pjrt.so")
        m = types.ModuleType("antenv.axon_hooks")
        m.get_axon_ntff_profile_hook = lambda: hook
        m.set_axon_ntff_profile_hook = lambda h: None
        sys.modules["antenv.axon_hooks"] = m
        import antenv
        antenv.axon_hooks = m
        bass_utils.upload_artifacts = lambda d: d
    except Exception:
        pass


# ---------------------------------------------------------------- program
def build_program(dbg=False):
    nc = bacc.Bacc("TRN2", target_bir_lowering=False, debug=False,
                   num_devices=NCORES)

    def din(name, shape, dt=f32r):
        return nc.dram_tensor(name, list(shape), dt, kind="ExternalInput").ap()

    io = {}
    io["xeT_in"] = din("xeT_in", (H, NSCAN), bf16)
    io["xdT_in"] = din("xdT_in", (H, NSEQ), bf16)
    io["amaskT"] = din("amaskT", (NSEQ, Bc), f32)
    for name, shape in [
        ("w0", (H, H3)), ("u0", (H, H3)),
        ("w1", (H2, H3)), ("u1", (H, H3)),
        ("wxd", (H, H3)), ("whd", (H, H3)), ("ud", (H, H3)),
        ("wa", (H2, H)), ("wcc", (H2, H)), ("wch", (H, H)),
        ("fct", (H2, H)),
        ("bn0", (1, H)), ("bn1", (1, H)), ("bnd", (1, H)),
    ]:
        io[name] = din(name, shape, bf16)
    io["b0"] = din("b0", (P, 12), f32)
    io["b1"] = din("b1", (P, 12), f32)
    io["bd"] = din("bd", (P, 12), f32)
    io["fcb"] = din("fcb", (P, 4), f32)
    io["owt"] = din("owt", (H, V), bf16)
    io["out"] = nc.dram_tensor("out", [NSEQ, V], bf16,
                               kind="ExternalOutput").ap()
    io["dbg"] = dbg
    if dbg:
        io["dbg_l0T"] = nc.dram_tensor("dbg_l0T", [P, 8, NSCAN], bf16,
                                       kind="ExternalOutput").ap()
        io["dbg_hencT"] = nc.dram_tensor("dbg_hencT", [P, 8, NSEQ], bf16,
                                         kind="ExternalOutput").ap()
        io["dbg_h0T"] = nc.dram_tensor("dbg_h0T", [P, 4, Bc], bf16,
                                       kind="ExternalOutput").ap()
        io["dbg_htT"] = nc.dram_tensor("dbg_htT", [P, 4, NSEQ], bf16,
                                       kind="ExternalOutput").ap()
        io["dbg_hall"] = nc.dram_tensor("dbg_hall", [P, 4, NSEQ], bf16,
                                        kind="ExternalOutput").ap()
        io["dbg_sc"] = nc.dram_tensor("dbg_sc", [P, 3, NSEQ], f32,
                                      kind="ExternalOutput").ap()

    with tile.TileContext(nc) as tc:
        _emit(nc, tc, io)
    nc.compile()
    return nc


def _emit(nc, tc, io):
    # ---------------- pools
    cpool_cm = tc.tile_pool(name="const", bufs=1)
    spool_cm = tc.tile_pool(name="state", bufs=2)
    gpool_cm = tc.tile_pool(name="gch", bufs=3)      # chain temporaries
    big_cm = tc.tile_pool(name="big", bufs=1)        # big sequence tiles
    wseq_cm = tc.tile_pool(name="wseq", bufs=1)      # phase-scoped weights
    prp_cm = tc.tile_pool(name="proj", bufs=4)
    pbank_cm = tc.tile_pool(name="pbank", bufs=1, space="PSUM")
    ppj_cm = tc.tile_pool(name="ppj", bufs=2, space="PSUM")
    cpool = cpool_cm.__enter__()
    spool = spool_cm.__enter__()
    gpool = gpool_cm.__enter__()
    big = big_cm.__enter__()
    wseq = wseq_cm.__enter__()
    prp = prp_cm.__enter__()
    pbank = pbank_cm.__enter__()
    ppj = ppj_cm.__enter__()

    # ---------------- constants
    identf = cpool.tile([P, P], f32)
    make_identity(nc, identf[:])
    identb = cpool.tile([P, P], bf16)
    nc.vector.tensor_copy(identb[:], identf[:])
    ones_c = cpool.tile([1, P], bf16)       # lhsT for bcast (K=1, M=128)
    nc.vector.memset(ones_c[:], 1.0)
    ones_r = cpool.tile([P, 1], bf16)       # lhsT for col-sum (K=128, M=1)
    nc.vector.memset(ones_r[:], 1.0)
    ones16 = cpool.tile([1, 16], bf16)      # rhs for bias injects
    nc.vector.memset(ones16[:], 1.0)

    def load(pool, name, shape, dt, tag, rearr=None):
        t = pool.tile(list(shape), dt, tag=tag, name=name + "_sb")
        src = io[name]
        if rearr is not None:
            src = src.rearrange(rearr, p=P)
            nc.sync.dma_start(t[:, :, :shape[2]] if len(shape) == 3 else t[:],
                              src)
        else:
            nc.sync.dma_start(t[:], src[:])
        return t

    bn0_sb = load(cpool, "bn0", (1, H), bf16, "bn0")
    bn1_sb = load(cpool, "bn1", (1, H), bf16, "bn1")
    bnd_sb = load(cpool, "bnd", (1, H), bf16, "bnd")
    b0_sb = load(cpool, "b0", (P, 12), f32, "b0")
    b1_sb = load(cpool, "b1", (P, 12), f32, "b1")
    bd_sb = load(cpool, "bd", (P, 12), f32, "bd")
    fcb_sb = load(cpool, "fcb", (P, 4), f32, "fcb")
    amask_sb = load(cpool, "amaskT", (P, 3, Bc), f32, "amaskT",
                    "(c p) b -> p c b")

    # PSUM banks (each padded to a full 2KB bank)
    ps_r = pbank.tile([P, 4, 16], f32, padded_shape=[P, 4, P], tag="bk_r")
    ps_z = pbank.tile([P, 4, 16], f32, padded_shape=[P, 4, P], tag="bk_z")
    ps_n = pbank.tile([P, 4, 32], f32, padded_shape=[P, 4, P], tag="bk_n")
    ps_sc = pbank.tile([P, 4, P], f32, tag="bk_sc")
    ps_ht = pbank.tile([P, 4, 16], f32, padded_shape=[P, 4, P], tag="bk_ht")
    ps_s = pbank.tile([P, 32], f32, padded_shape=[P, 512], tag="bk_s")

    def bank_mms(ps_g, nw, groups):
        """Emit per-region accumulation groups into one PSUM bank.

        groups: list of regions; each region is a list of (m, off, lhsT,
        rhs) mms accumulated into ps_g[:, m, off:off+nw].  Each region is a
        contiguous accumulation group (start on its first mm, stop on its
        last), the only pattern valid under both per-element (HW) and
        whole-bank (CoreSim) has_written-clear semantics."""
        for grp in groups:
            last = len(grp) - 1
            for i, (m, off, lh, rh) in enumerate(grp):
                nc.tensor.matmul(ps_g[:, m, off:off + nw], lh, rh,
                                 start=(i == 0), stop=(i == last),
                                 skip_group_check=True)

    def warm_mm(rhs):
        nc.tensor.matmul(ps_sc[:, 3, 0:P], identb[:], rhs,
                         start=True, stop=True, skip_group_check=True)

    # ---------------- pre-projection GEMM -> bf16 xp tile (+ bias)
    def pre_gemm(xp_t, w_sb, ko, rhs, ncols, bias_sb):
        segs = [(s, min(384, ncols - s)) for s in range(0, ncols, 384)]
        for c in range(12):
            for s0, sw in segs:
                pp = ppj.tile([P, 512], f32, tag="gemm", name="pp")
                for k in range(ko):
                    nc.tensor.matmul(pp[:, :sw],
                                     w_sb[:, k, c * P:(c + 1) * P],
                                     rhs[:, k, s0:s0 + sw],
                                     start=(k == 0), stop=(k == ko - 1))
                nc.scalar.activation(xp_t[:, c, s0:s0 + sw], pp[:, :sw],
                                     AF.Identity, bias=bias_sb[:, c:c + 1])

    # ================================================= input staging
    xeT = load(big, "xeT_in", (P, 4, NSCAN), bf16, "xeT", "(k p) n -> p k n")
    xdT = load(big, "xdT_in", (P, 4, NSEQ), bf16, "xdT", "(k p) n -> p k n")

    w0_sb = load(wseq, "w0", (P, 4, H3), bf16, "wqA", "(k p) n -> p k n")
    wxd_sb = load(wseq, "wxd", (P, 4, H3), bf16, "wqB", "(k p) n -> p k n")

    xp0 = big.tile([P, 12, NSCAN], bf16, tag="xp0")
    xpx = big.tile([P, 12, NSEQ], bf16, tag="xpx")
    pre_gemm(xp0, w0_sb, 4, xeT, NSCAN, b0_sb)
    pre_gemm(xpx, wxd_sb, 4, xdT, NSEQ, bd_sb)

    l0T = big.tile([P, 8, NSCAN], bf16, tag="l0T")
    hencT = big.tile([P, 8, NSEQ], bf16, tag="hencT")

    # ================================================= GRU scan (encoder)
    def enc_scan(u_sb, bn_sb, xp_t, emit):
        h = None
        for s in range(T):
            c0 = s * 16
            for g, ps_g in ((0, ps_r), (2, ps_n), (1, ps_z)):
                groups = []
                for m in range(4):
                    c = g * 4 + m
                    grp = []
                    if g == 2:
                        grp.append((m, 0, bn_sb[:, m * P:(m + 1) * P],
                                    ones16[:, :]))
                    else:
                        grp.append((m, 0, identb[:], xp_t[:, c, c0:c0 + 16]))
                    if h is not None:
                        for k in range(4):
                            grp.append((m, 0, u_sb[:, k, c * P:(c + 1) * P],
                                        h[:, k, :]))
                    groups.append(grp)
                bank_mms(ps_g, 16, groups)
            warm_mm(xp_t[:, 0, 0:P])
            # ---- chain
            r = gpool.tile([P, 4, 16], f32, tag="gr", name="r")
            nc.scalar.activation(r[:], ps_r[:, :, 0:16], AF.Sigmoid)
            u_t = gpool.tile([P, 4, 16], f32, tag="gu", name="u_t")
            nc.vector.tensor_mul(u_t[:], r[:], ps_n[:, :, 0:16])
            v_t = gpool.tile([P, 4, 16], f32, tag="gv", name="v_t")
            nc.vector.tensor_add(v_t[:], u_t[:], xp_t[:, 8:12, c0:c0 + 16])
            n_t = gpool.tile([P, 4, 16], f32, tag="gn", name="n_t")
            nc.scalar.activation(n_t[:], v_t[:], AF.Tanh)
            z_t = gpool.tile([P, 4, 16], f32, tag="gz", name="z_t")
            nc.scalar.activation(z_t[:], ps_z[:, :, 0:16], AF.Sigmoid)
            h_new = spool.tile([P, 4, 16], bf16, tag="h", name="h_new")
            d_t = gpool.tile([P, 4, 16], f32, tag="gd", name="d_t")
            if h is not None:
                nc.vector.tensor_sub(d_t[:], h[:], n_t[:])
                e_t = gpool.tile([P, 4, 16], f32, tag="ge", name="e_t")
                nc.vector.tensor_mul(e_t[:], z_t[:], d_t[:])
                nc.vector.tensor_add(h_new[:], n_t[:], e_t[:])
            else:
                nc.vector.tensor_mul(d_t[:], z_t[:], n_t[:])
                nc.vector.tensor_sub(h_new[:], n_t[:], d_t[:])
            emit(s, h_new)
            h = h_new

    def emit_l0(s, h_new):
        sr = T - 1 - s
        nc.scalar.copy(out=l0T[:, 0:4, s * 16:s * 16 + 8],
                       in_=h_new[:, :, 0:8])
        nc.scalar.copy(out=l0T[:, 4:8, s * 16 + 8:s * 16 + 16],
                       in_=h_new[:, :, 8:16])
        nc.vector.tensor_copy(out=l0T[:, 0:4, sr * 16 + 8:sr * 16 + 16],
                              in_=h_new[:, :, 0:8])
        nc.vector.tensor_copy(out=l0T[:, 4:8, sr * 16:sr * 16 + 8],
                              in_=h_new[:, :, 8:16])

    def emit_henc(s, h_new):
        sr = T - 1 - s
        nc.scalar.copy(out=hencT[:, 0:4, s * Bc:(s + 1) * Bc],
                       in_=h_new[:, :, 0:8])
        nc.vector.tensor_copy(out=hencT[:, 4:8, sr * Bc:(sr + 1) * Bc],
                              in_=h_new[:, :, 8:16])

    u0_sb = load(wseq, "u0", (P, 4, H3), bf16, "wqA", "(k p) n -> p k n")
    enc_scan(u0_sb, bn0_sb, xp0, emit_l0)

    w1_sb = load(wseq, "w1", (P, 8, H3), bf16, "wqC", "(k p) n -> p k n")
    xp1 = big.tile([P, 12, NSCAN], bf16, tag="xp0", name="xp1")
    pre_gemm(xp1, w1_sb, 8, l0T, NSCAN, b1_sb)

    u1_sb = load(wseq, "u1", (P, 4, H3), bf16, "wqA", "(k p) n -> p k n")
    enc_scan(u1_sb, bn1_sb, xp1, emit_henc)

    if io["dbg"]:
        nc.sync.dma_start(io["dbg_l0T"][:], l0T[:])
        nc.sync.dma_start(io["dbg_hencT"][:], hencT[:])

    # ================================================= attention precompute
    wa_sb = load(wseq, "wa", (P, 8, H), bf16, "wqD", "(k p) n -> p k n")
    gT = big.tile([P, 4, NSEQ], bf16, tag="gT")
    for m in range(4):
        pp = ppj.tile([P, 512], f32, tag="gemm", name="pp")
        for k in range(8):
            nc.tensor.matmul(pp[:, :NSEQ], wa_sb[:, k, m * P:(m + 1) * P],
                             hencT[:, k, :], start=(k == 0), stop=(k == 7))
        nc.scalar.copy(out=gT[:, m, :], in_=pp[:, :NSEQ])

    wcc_sb = load(wseq, "wcc", (P, 8, H), bf16, "wqD", "(k p) n -> p k n")
    pf = big.tile([P, 3, H], bf16, tag="pf")
    for m in range(3):
        pp = ppj.tile([P, 512], f32, tag="gemm", name="pp")
        for k in range(8):
            nc.tensor.matmul(pp[:, :H], hencT[:, k, m * P:(m + 1) * P],
                             wcc_sb[:, k, :], start=(k == 0), stop=(k == 7))
        nc.vector.tensor_copy(out=pf[:, m, :], in_=pp[:, :H])

    # h0 = tanh(fc_init([hf; hb]) + fcb)
    fct_sb = load(wseq, "fct", (P, 8, H), bf16, "wqD", "(k p) n -> p k n")
    groups = []
    for m in range(4):
        grp = []
        for k in range(8):
            rhs = (hencT[:, k, (T - 1) * Bc:T * Bc] if k < 4
                   else hencT[:, k, 0:Bc])
            grp.append((m, 0, fct_sb[:, k, m * P:(m + 1) * P], rhs))
        groups.append(grp)
    bank_mms(ps_ht, Bc, groups)
    h0T = spool.tile([P, 4, Bc], bf16, tag="hd", name="h0T")
    for m in range(4):
        nc.scalar.activation(h0T[:, m, :], ps_ht[:, m, 0:Bc], AF.Tanh,
                             bias=fcb_sb[:, m:m + 1])

    # ================================================= decoder
    ud_sb = load(wseq, "ud", (P, 4, H3), bf16, "wqA", "(k p) n -> p k n")
    whd_sb = load(wseq, "whd", (P, 4, H3), bf16, "wqB", "(k p) n -> p k n")
    wch_sb = load(wseq, "wch", (P, 4, H), bf16, "wqE", "(k p) n -> p k n")
    htall = big.tile([P, 4, NSEQ], bf16, tag="htall")
    owt_r = io["owt"].rearrange("(k p) v -> p k v", p=P)

    def emit_proj(m, c0, cw):
        ow = prp.tile([P, 4, 512], bf16, tag="ow", name="ow")
        nc.sync.dma_start(ow[:, 0:2, :cw], owt_r[:, 0:2, c0:c0 + cw])
        nc.sync.dma_start(ow[:, 2:4, :cw], owt_r[:, 2:4, c0:c0 + cw])
        pp = ppj.tile([P, 512], f32, tag="gemm", name="pp")
        for k in range(4):
            nc.tensor.matmul(pp[:, :cw], htall[:, k, m * P:(m + 1) * P],
                             ow[:, k, :cw], start=(k == 0), stop=(k == 3))
        ob = prp.tile([P, 512], bf16, tag="ob", name="ob")
        nc.scalar.copy(out=ob[:, :cw], in_=pp[:, :cw])
        nc.scalar.dma_start(io["out"][m * P:(m + 1) * P, c0:c0 + cw],
                            ob[:, :cw])

    todo = {m: list(VCHUNKS) for m in range(3)}
    if io["dbg"]:
        hall_d = big.tile([P, 4, NSEQ], bf16, tag="halld")
        scall_d = big.tile([P, 3, NSEQ], f32, tag="scalld")

    hT = h0T
    htT = None
    for t in range(T):
        c0 = t * Bc
        # ---- gates
        for g, ps_g in ((0, ps_r), (2, ps_n), (1, ps_z)):
            groups = []
            if g == 2:
                # nx (outside r*(.)) regions first: x-part inject + ht-part
                for m in range(4):
                    c = g * 4 + m
                    grp = [(m, 16, identb[:], xpx[:, c, c0:c0 + Bc])]
                    if htT is not None:
                        for k in range(4):
                            grp.append((m, 16,
                                        whd_sb[:, k, c * P:(c + 1) * P],
                                        htT[:, k, :]))
                    groups.append(grp)
            for m in range(4):
                c = g * 4 + m
                if g == 2:
                    grp = [(m, 0, bnd_sb[:, m * P:(m + 1) * P],
                            ones16[:, 0:Bc])]
                else:
                    grp = [(m, 0, identb[:], xpx[:, c, c0:c0 + Bc])]
                for k in range(4):
                    grp.append((m, 0, ud_sb[:, k, c * P:(c + 1) * P],
                                hT[:, k, :]))
                if htT is not None and g != 2:
                    for k in range(4):
                        grp.append((m, 0, whd_sb[:, k, c * P:(c + 1) * P],
                                    htT[:, k, :]))
                groups.append(grp)
            bank_mms(ps_g, Bc, groups)
        # ---- GRU chain (tanh-sigmoid keeps decoder on the exp/tanh set)
        thr = gpool.tile([P, 4, Bc], f32, tag="gr", name="thr")
        nc.scalar.activation(thr[:], ps_r[:, :, 0:Bc], AF.Tanh, scale=0.5)
        u_t = gpool.tile([P, 4, Bc], f32, tag="gu", name="u_t")
        nc.vector.scalar_tensor_tensor(u_t[:], thr[:], 1.0,
                                       ps_n[:, :, 0:Bc], OP.add, OP.mult)
        v_t = gpool.tile([P, 4, Bc], f32, tag="gv", name="v_t")
        nc.vector.scalar_tensor_tensor(v_t[:], u_t[:], 0.5,
                                       ps_n[:, :, 16:16 + Bc],
                                       OP.mult, OP.add)
        n_t = gpool.tile([P, 4, Bc], f32, tag="gn", name="n_t")
        nc.scalar.activation(n_t[:], v_t[:], AF.Tanh)
        thz = gpool.tile([P, 4, Bc], f32, tag="gz", name="thz")
        nc.scalar.activation(thz[:], ps_z[:, :, 0:Bc], AF.Tanh, scale=0.5)
        d_t = gpool.tile([P, 4, Bc], f32, tag="gd", name="d_t")
        nc.vector.tensor_sub(d_t[:], hT[:], n_t[:])
        e_t = gpool.tile([P, 4, Bc], f32, tag="ge", name="e_t")
        nc.vector.scalar_tensor_tensor(e_t[:], thz[:], 1.0, d_t[:],
                                       OP.add, OP.mult)
        h_new = spool.tile([P, 4, Bc], bf16, tag="hd", name="h_new")
        nc.vector.scalar_tensor_tensor(h_new[:], e_t[:], 0.5, n_t[:],
                                       OP.mult, OP.add)
        hT = h_new
        if io["dbg"]:
            nc.vector.tensor_copy(hall_d[:, :, c0:c0 + Bc], h_new[:])
        # ---- attention: scores (ps_sc), wch part (ps_ht, early)
        groups = []
        for mt in range(3):
            groups.append([(mt, 0, gT[:, k, mt * P:(mt + 1) * P],
                            hT[:, k, :]) for k in range(4)])
        bank_mms(ps_sc, Bc, groups)
        # ht bank: per-m contiguous groups [wch k0..3, ctx j0..2]; the
        # ctx part is emitted below once alphaT exists
        scm = gpool.tile([P, 3, Bc], f32, tag="gsc", name="scm")
        nc.vector.tensor_add(scm[:], ps_sc[:, 0:3, 0:Bc], amask_sb[:])
        if io["dbg"]:
            nc.vector.tensor_copy(scall_d[:, :, c0:c0 + Bc], scm[:])
        expT = gpool.tile([P, 3, Bc], bf16, tag="gexp", name="expT")
        nc.scalar.activation(expT[:], scm[:], AF.Exp)
        # S = sum_t exp (per example), via ones col-sum matmuls
        for j in range(3):
            nc.tensor.matmul(ps_s[0:1, 0:Bc], ones_r[:],
                             expT[:, j, :], start=(j == 0), stop=(j == 2),
                             skip_group_check=True)
        rS = gpool.tile([1, Bc], bf16, tag="grs", name="rS")
        with nc.allow_low_precision(reason="bf16 1/S is plenty for softmax"):
            nc.vector.reciprocal(rS[:], ps_s[0:1, 0:Bc])
        # broadcast 1/S across partitions via PE
        nc.tensor.matmul(ps_s[:, 16:16 + Bc], ones_c[:], rS[:],
                         start=True, stop=True, skip_group_check=True)
        alphaT = gpool.tile([P, 3, Bc], bf16, tag="galp", name="alphaT")
        for j in range(3):
            nc.vector.tensor_mul(alphaT[:, j, :], expT[:, j, :],
                                 ps_s[:, 16:16 + Bc])
        groups = []
        for m in range(4):
            grp = [(m, 0, wch_sb[:, k, m * P:(m + 1) * P], hT[:, k, :])
                   for k in range(4)]
            grp += [(m, 0, pf[:, j, m * P:(m + 1) * P], alphaT[:, j, :])
                    for j in range(3)]
            groups.append(grp)
        bank_mms(ps_ht, Bc, groups)
        ht_new = spool.tile([P, 4, Bc], bf16, tag="htd", name="ht_new")
        nc.scalar.activation(ht_new[:], ps_ht[:, :, 0:Bc], AF.Tanh)
        nc.vector.tensor_copy(out=htall[:, :, c0:c0 + Bc], in_=ht_new[:])
        htT = ht_new
        # ---- interleaved vocab projection
        if t >= 17:
            m = min(2, (t - 17) // 16)
            for _ in range(5):
                if todo[m]:
                    cc0, cw = todo[m].pop(0)
                    emit_proj(m, cc0, cw)

    for m in range(3):
        while todo[m]:
            cc0, cw = todo[m].pop(0)
            emit_proj(m, cc0, cw)

    if io["dbg"]:
        nc.sync.dma_start(io["dbg_h0T"][:], h0T[:])
        nc.sync.dma_start(io["dbg_htT"][:], htall[:])
        nc.sync.dma_start(io["dbg_hall"][:], hall_d[:])
        nc.sync.dma_start(io["dbg_sc"][:], scall_d[:])

    for cm in (ppj_cm, pbank_cm, prp_cm, wseq_cm, big_cm, gpool_cm,
               spool_cm, cpool_cm):
        cm.__exit__(None, None, None)


# ---------------------------------------------------------------- host side
_PROGRAM = None


def _get_program():
    global _PROGRAM
    if _PROGRAM is None:
        _install_profile_hook()
        _PROGRAM = build_program()
    return _PROGRAM


def _prep_shared(inputs):
    import ml_dtypes
    f = np.float32
    bf = ml_dtypes.bfloat16

    def cat_u(pre):
        return np.concatenate(
            [np.asarray(inputs[f"{pre}_Ur"], f).T,
             np.asarray(inputs[f"{pre}_Uz"], f).T,
             np.asarray(inputs[f"{pre}_Un"], f).T], axis=1)

    g = {}
    g["w0"] = np.asarray(inputs["enc0_Wih"], f).T.astype(bf)
    g["u0"] = cat_u("enc0").astype(bf)
    g["w1"] = np.asarray(inputs["enc1_Wih"], f).T.astype(bf)
    g["u1"] = cat_u("enc1").astype(bf)
    dwih = np.asarray(inputs["dec_Wih"], f)
    g["wxd"] = dwih[:, :H].T.astype(bf)
    g["whd"] = dwih[:, H:].T.astype(bf)
    g["ud"] = cat_u("dec").astype(bf)
    scale = np.float32(1.0) / np.sqrt(np.float32(H2))
    g["wa"] = (np.asarray(inputs["Wa"], f) * scale).astype(bf)
    acw = np.asarray(inputs["attn_combine_w"], f)
    g["wch"] = acw[:, :H].T.astype(bf)
    g["wcc"] = acw[:, H:].T.astype(bf)
    g["fct"] = np.asarray(inputs["fc_init_w"], f).T.astype(bf)
    g["bn0"] = np.asarray(inputs["enc0_bn"], f)[None, :].astype(bf)
    g["bn1"] = np.asarray(inputs["enc1_bn"], f)[None, :].astype(bf)
    g["bnd"] = np.asarray(inputs["dec_bn"], f)[None, :].astype(bf)
    g["b0"] = np.asarray(inputs["enc0_bih"], f).reshape(12, P).T
    g["b1"] = np.asarray(inputs["enc1_bih"], f).reshape(12, P).T
    g["bd"] = np.asarray(inputs["dec_bih"], f).reshape(12, P).T
    g["fcb"] = np.asarray(inputs["fc_init_b"], f).reshape(4, P).T
    g["owt"] = np.asarray(inputs["out_w"], f).T.astype(bf)
    for k in g:
        g[k] = np.ascontiguousarray(g[k])
    return g


def _prep_core(inputs, c):
    src = np.asarray(inputs["src"])
    tgt = np.asarray(inputs["tgt"])
    emb = np.asarray(inputs["emb"], np.float32)
    import ml_dtypes
    bf = ml_dtypes.bfloat16
    si = src[:, c * Bc:(c + 1) * Bc].astype(np.int64)      # (48, 8)
    ti = tgt[:, c * Bc:(c + 1) * Bc].astype(np.int64)
    idx_enc = np.empty((T, 2, Bc), np.int64)
    idx_enc[:, 0, :] = si
    idx_enc[:, 1, :] = si[::-1]
    xeT_in = np.ascontiguousarray(emb[idx_enc.reshape(NSCAN)].T.astype(bf))
    xdT_in = np.ascontiguousarray(emb[ti.reshape(NSEQ)].T.astype(bf))
    m = np.full((T, Bc, Bc), NEG, np.float32)
    for b in range(Bc):
        m[:, b, b] = np.where(si[:, b] != 0, np.float32(0.0),
                              np.float32(NEG))
    return {"xeT_in": xeT_in,
            "xdT_in": xdT_in,
            "amaskT": np.ascontiguousarray(m.reshape(NSEQ, Bc))}


def kernel(**inputs):
    nc = _get_program()
    shared = _prep_shared(inputs)
    in_maps = []
    for c in range(NCORES):
        im = dict(shared)
        im.update(_prep_core(inputs, c))
        in_maps.append(im)
    res = run_bass_kernel_spmd(nc, in_maps, core_ids=list(range(NCORES)))
    logits = np.empty((T, B, V), np.float32)
    for c in range(NCORES):
        logits[:, c * Bc:(c + 1) * Bc, :] = \
            res.results[c]["out"].astype(np.float32).reshape(T, Bc, V)
    return logits
